# revision 1
# baseline (speedup 1.0000x reference)
"""Trainium2 Bass kernel for nn_H2GT_HGNN: two HGNN convs + single-head GAT +
gated attention pooling, sharded across 8 NeuronCores.

Strategy (dst-node row sharding, core c owns rows R_c = c*1024..(c+1)*1024):
  - Host passes G^T (bf16) so both G @ X convs read contiguous column blocks
    and feed the PE array as natural [K=u, M]/[K=u, N] operands.
  - GAT edge softmax is reformulated dense + rank-1:
        exp(leaky(es_u + ed_v)) = eu_u * max(ev_v, r_u * ev2_v)
    with eu=exp(es), ev=exp(ed), r=exp(-0.8 es), ev2=exp(0.2 ed).  The edge
    multiplicity mask C[u,v] (built on host from src/dst, + self loops) makes
        W[u,v] = C[u,v] * eu_u * max(ev_v, r_u*ev2_v)
    so the whole scatter/softmax/aggregate is 2 DVE passes per tile plus a
    dense masked matmul [out|denom] = W^T @ [z|1] on the PE.  No max-shift is
    needed: |es|,|ed| < 1 for this model family (verified; exp never overflows
    in fp32 accumulation).
  - Collectives: AllGather of XW1 / HW2 / [z|1] / [es,ed], final AllReduce of
    the pooled [1,129] numerator|denominator.
"""

import os
import numpy as np
import ml_dtypes

BF16 = ml_dtypes.bfloat16

# Problem sizes (hardcoded per contract)
N, F_IN, HID, OUT, D_ATT, E = 8192, 512, 256, 128, 64, 262144
M = 8          # cores
P = 128        # partitions
R = N // M     # rows per core (1024)
K = N // P     # contraction chunks (64)
RT = R // P    # row tiles per core (8)
O1 = OUT + 1   # z plus ones column


def build_program():
    import concourse.bass as bass
    import concourse.tile as tile
    from concourse import bacc, mybir
    from contextlib import ExitStack

    dt = mybir.dt
    ALU = mybir.AluOpType
    ACT = mybir.ActivationFunctionType

    nc = bacc.Bacc(None, target_bir_lowering=False, debug=False)

    # ---- I/O ----
    gt = nc.dram_tensor("gt", [N, R], dt.bfloat16, kind="ExternalInput")
    ct = nc.dram_tensor("ct", [N, R], dt.bfloat16, kind="ExternalInput")
    xt = nc.dram_tensor("xt", [F_IN, R], dt.bfloat16, kind="ExternalInput")
    w1 = nc.dram_tensor("w1", [F_IN, HID], dt.bfloat16, kind="ExternalInput")
    w2 = nc.dram_tensor("w2", [HID, OUT], dt.bfloat16, kind="ExternalInput")
    wg = nc.dram_tensor("wg", [OUT, OUT], dt.bfloat16, kind="ExternalInput")
    vab = nc.dram_tensor("vab", [OUT, 2], dt.bfloat16, kind="ExternalInput")
    wab = nc.dram_tensor("wab", [OUT, 2 * D_ATT], dt.bfloat16, kind="ExternalInput")
    b1bc = nc.dram_tensor("b1bc", [P, HID], dt.float32, kind="ExternalInput")
    b2bc = nc.dram_tensor("b2bc", [P, OUT], dt.float32, kind="ExternalInput")
    babbc = nc.dram_tensor("babbc", [P, 2 * D_ATT], dt.float32, kind="ExternalInput")
    wcbc = nc.dram_tensor("wcbc", [P, D_ATT], dt.float32, kind="ExternalInput")
    bcbc = nc.dram_tensor("bcbc", [P, 1], dt.float32, kind="ExternalInput")
    ident = nc.dram_tensor("ident", [P, P], dt.bfloat16, kind="ExternalInput")
    out_ext = nc.dram_tensor("out", [1, OUT], dt.float32, kind="ExternalOutput")

    groups = [list(range(M))]

    with tile.TileContext(nc) as tc, ExitStack() as ctx:
        dram = ctx.enter_context(tc.tile_pool(name="dram", bufs=1, space="DRAM"))
        xw1b = dram.tile([R, HID], dt.bfloat16)
        xw1f = dram.tile([N, HID], dt.bfloat16, addr_space="Shared")
        hw2b = dram.tile([R, OUT], dt.bfloat16)
        hw2f = dram.tile([N, OUT], dt.bfloat16, addr_space="Shared")
        zb = dram.tile([R, O1], dt.bfloat16)
        zf = dram.tile([N, O1], dt.bfloat16, addr_space="Shared")
        eb = dram.tile([R, 2], dt.float32)
        ef = dram.tile([N, 2], dt.float32, addr_space="Shared")
        poolb = dram.tile([1, O1], dt.float32)
        poolr = dram.tile([1, O1], dt.float32, addr_space="Shared")

        const = ctx.enter_context(tc.tile_pool(name="const", bufs=1))
        big = ctx.enter_context(tc.tile_pool(name="big", bufs=1))

        # ---- resident constants ----
        w1_sb = const.tile([P, (F_IN // P) * HID], dt.bfloat16)
        nc.sync.dma_start(w1_sb.rearrange("p (k h) -> p k h", h=HID), w1.rearrange("(k p) h -> p k h", p=P))
        w2_sb = const.tile([P, (HID // P) * OUT], dt.bfloat16)
        nc.sync.dma_start(w2_sb.rearrange("p (k h) -> p k h", h=OUT), w2.rearrange("(k p) h -> p k h", p=P))
        wg_sb = const.tile([P, OUT], dt.bfloat16)
        nc.sync.dma_start(wg_sb[:], wg[:])
        vab_sb = const.tile([P, 2], dt.bfloat16)
        nc.sync.dma_start(vab_sb[:], vab[:])
        wab_sb = const.tile([P, 2 * D_ATT], dt.bfloat16)
        nc.sync.dma_start(wab_sb[:], wab[:])
        b1bc_sb = const.tile([P, HID], dt.float32)
        nc.sync.dma_start(b1bc_sb[:], b1bc[:])
        b2bc_sb = const.tile([P, OUT], dt.float32)
        nc.sync.dma_start(b2bc_sb[:], b2bc[:])
        babbc_sb = const.tile([P, 2 * D_ATT], dt.float32)
        nc.sync.dma_start(babbc_sb[:], babbc[:])
        wcbc_sb = const.tile([P, D_ATT], dt.float32)
        nc.sync.dma_start(wcbc_sb[:], wcbc[:])
        bcbc_sb = const.tile([P, 1], dt.float32)
        nc.sync.dma_start(bcbc_sb[:], bcbc[:])
        ident_sb = const.tile([P, P], dt.bfloat16)
        nc.sync.dma_start(ident_sb[:], ident[:])
        ones_sb = const.tile([1, P], dt.float32)
        nc.vector.memset(ones_sb[:], 1.0)

        # ================= phase A: XW1 rows R_c, then AllGather =============
        with tc.tile_pool(name="phA", bufs=1) as pa, \
             tc.tile_pool(name="phA_ps", bufs=1, space="PSUM") as pa_ps:
            xt_sb = pa.tile([P, (F_IN // P) * R], dt.bfloat16)
            nc.sync.dma_start(xt_sb.rearrange("p (k r) -> p k r", r=R), xt.rearrange("(k p) r -> p k r", p=P))
            for rt in range(RT):
                ps = pa_ps.tile([P, HID], dt.float32, tag="pa", bufs=2)
                for k in range(F_IN // P):
                    nc.tensor.matmul(
                        ps[:],
                        xt_sb[:, k * R + rt * P : k * R + (rt + 1) * P],
                        w1_sb[:, k * HID : (k + 1) * HID],
                        start=(k == 0), stop=(k == F_IN // P - 1))
                xw1_sb = pa.tile([P, HID], dt.bfloat16, tag="xw1s", bufs=2)
                nc.vector.tensor_tensor(xw1_sb[:], ps[:], b1bc_sb[:], op=ALU.add)
                nc.sync.dma_start(xw1b[rt * P : (rt + 1) * P, :], xw1_sb[:])
        nc.gpsimd.collective_compute(
            "AllGather", ALU.bypass, replica_groups=groups,
            ins=[xw1b[:]], outs=[xw1f[:]])

        # ================= phase B1: hT = relu(G @ XW1)^T ====================
        xw1f_sb = big.tile([P, K * HID], dt.bfloat16)
        nc.sync.dma_start(xw1f_sb.rearrange("p (k h) -> p k h", h=HID), xw1f.rearrange("(k p) h -> p k h", p=P))
        hT_sb = big.tile([P, 2 * R], dt.bfloat16)  # [hid=256 over 2 tiles, r=1024]
        with tc.tile_pool(name="phB1", bufs=3) as pb, \
             tc.tile_pool(name="phB1_ps", bufs=1, space="PSUM") as pb_ps:
            hps = [pb_ps.tile([P, 512], dt.float32, tag=f"h{j}{n}", bufs=1, name=f"hps{j}{n}")
                   for j in range(2) for n in range(2)]
            for k in range(K):
                gt_t = pb.tile([P, R], dt.bfloat16, tag="gt")
                nc.sync.dma_start(gt_t[:], gt[k * P : (k + 1) * P, :])
                for j in range(2):
                    for n in range(2):
                        nc.tensor.matmul(
                            hps[2 * j + n][:],
                            xw1f_sb[:, k * HID + j * P : k * HID + (j + 1) * P],
                            gt_t[:, n * 512 : (n + 1) * 512],
                            start=(k == 0), stop=(k == K - 1))
            for j in range(2):
                for n in range(2):
                    nc.scalar.activation(
                        hT_sb[:, j * R + n * 512 : j * R + (n + 1) * 512],
                        hps[2 * j + n][:], ACT.Relu)

        # ---- HW2 rows R_c = hT^T @ W2 + b2, AllGather ----
        with tc.tile_pool(name="phB1b", bufs=1) as pbb, \
             tc.tile_pool(name="phB1b_ps", bufs=1, space="PSUM") as pbb_ps:
            for rt in range(RT):
                ps2 = pbb_ps.tile([P, OUT], dt.float32, tag="hw2", bufs=2)
                for j in range(2):
                    nc.tensor.matmul(
                        ps2[:],
                        hT_sb[:, j * R + rt * P : j * R + (rt + 1) * P],
                        w2_sb[:, j * OUT : (j + 1) * OUT],
                        start=(j == 0), stop=(j == 1))
                hw2_sb = pbb.tile([P, OUT], dt.bfloat16, tag="hw2s", bufs=2)
                nc.vector.tensor_tensor(hw2_sb[:], ps2[:], b2bc_sb[:], op=ALU.add)
                nc.sync.dma_start(hw2b[rt * P : (rt + 1) * P, :], hw2_sb[:])
        nc.gpsimd.collective_compute(
            "AllGather", ALU.bypass, replica_groups=groups,
            ins=[hw2b[:]], outs=[hw2f[:]])

        # ================= phase B2: h2T = (G @ HW2)^T, z, es/ed =============
        hw2f_sb = big.tile([P, K * OUT], dt.bfloat16)
        nc.sync.dma_start(hw2f_sb.rearrange("p (k h) -> p k h", h=OUT), hw2f.rearrange("(k p) h -> p k h", p=P))
        h2T_sb = big.tile([P, R], dt.bfloat16)
        with tc.tile_pool(name="phB2", bufs=3) as pc, \
             tc.tile_pool(name="phB2_ps", bufs=1, space="PSUM") as pc_ps:
            h2ps = [pc_ps.tile([P, 512], dt.float32, tag=f"h2{n}", bufs=1, name=f"h2ps{n}")
                    for n in range(2)]
            for k in range(K):
                gt_t = pc.tile([P, R], dt.bfloat16, tag="gt2")
                nc.sync.dma_start(gt_t[:], gt[k * P : (k + 1) * P, :])
                for n in range(2):
                    nc.tensor.matmul(
                        h2ps[n][:],
                        hw2f_sb[:, k * OUT : k * OUT + P],
                        gt_t[:, n * 512 : (n + 1) * 512],
                        start=(k == 0), stop=(k == K - 1))
            for n in range(2):
                nc.scalar.activation(
                    h2T_sb[:, n * 512 : (n + 1) * 512], h2ps[n][:], ACT.Copy)

        with tc.tile_pool(name="phB2b", bufs=1) as pd, \
             tc.tile_pool(name="phB2b_ps", bufs=1, space="PSUM") as pd_ps:
            for rt in range(RT):
                zps = pd_ps.tile([P, OUT], dt.float32, tag="z", bufs=2)
                nc.tensor.matmul(
                    zps[:], h2T_sb[:, rt * P : (rt + 1) * P], wg_sb[:],
                    start=True, stop=True)
                eps = pd_ps.tile([P, 2], dt.float32, tag="e", bufs=2)
                nc.tensor.matmul(
                    eps[:], h2T_sb[:, rt * P : (rt + 1) * P], vab_sb[:],
                    start=True, stop=True)
                zext_sb = pd.tile([P, O1], dt.bfloat16, tag="zx", bufs=2)
                nc.scalar.activation(zext_sb[:, 0:OUT], zps[:], ACT.Copy)
                nc.vector.memset(zext_sb[:, OUT:O1], 1.0)
                esed_sb = pd.tile([P, 2], dt.float32, tag="es", bufs=2)
                nc.vector.tensor_copy(esed_sb[:], eps[:])
                nc.sync.dma_start(zb[rt * P : (rt + 1) * P, :], zext_sb[:])
                nc.sync.dma_start(eb[rt * P : (rt + 1) * P, :], esed_sb[:])
        nc.gpsimd.collective_compute(
            "AllGather", ALU.bypass, replica_groups=groups,
            ins=[zb[:]], outs=[zf[:]])
        nc.gpsimd.collective_compute(
            "AllGather", ALU.bypass, replica_groups=groups,
            ins=[eb[:]], outs=[ef[:]])

        # ================= phase D: attention ================================
        zf_sb = big.tile([P, K * O1], dt.bfloat16)
        nc.sync.dma_start(zf_sb.rearrange("p (k h) -> p k h", h=O1), zf.rearrange("(k p) h -> p k h", p=P))
        est_sb = big.tile([P, 2 * K], dt.float32)
        nc.sync.dma_start(est_sb.rearrange("p (k j) -> p k j", j=2), ef.rearrange("(k p) j -> p k j", p=P))
        eu_sb = big.tile([P, 2 * K], dt.bfloat16)
        nc.scalar.activation(eu_sb[:], est_sb[:], ACT.Exp, scale=1.0)
        r_sb = big.tile([P, 2 * K], dt.bfloat16)
        nc.scalar.activation(r_sb[:], est_sb[:], ACT.Exp, scale=-0.8)

        # local ed row -> broadcast -> ev, ev2
        edrow_sb = big.tile([1, R], dt.float32)
        nc.sync.dma_start(edrow_sb[:], eb[:, 1:2].rearrange("r j -> j r"))
        ev_sb = big.tile([P, R], dt.bfloat16)
        ev2_sb = big.tile([P, R], dt.bfloat16)
        with tc.tile_pool(name="phD0_ps", bufs=1, space="PSUM") as pe_ps:
            for n in range(2):
                bc_ps = pe_ps.tile([P, 512], dt.float32, tag="bc", bufs=2)
                nc.tensor.matmul(
                    bc_ps[:], ones_sb[:], edrow_sb[:, n * 512 : (n + 1) * 512],
                    start=True, stop=True)
                nc.scalar.activation(
                    ev_sb[:, n * 512 : (n + 1) * 512], bc_ps[:], ACT.Exp, scale=1.0)
                nc.scalar.activation(
                    ev2_sb[:, n * 512 : (n + 1) * 512], bc_ps[:], ACT.Exp, scale=0.2)

        outex = [None] * RT
        with tc.tile_pool(name="phD", bufs=1) as pf, \
             tc.tile_pool(name="phDo", bufs=1) as pfo:
          with tc.tile_pool(name="phD_ps", bufs=1, space="PSUM") as pf_ps:
            att_ps = [pf_ps.tile([P, O1], dt.float32, tag=f"att{v}", bufs=1, name=f"attps{v}")
                      for v in range(RT)]
            for k in range(K):
                ct_t = pf.tile([P, R], dt.bfloat16, tag="ct", bufs=3)
                nc.sync.dma_start(ct_t[:], ct[k * P : (k + 1) * P, :])
                t_t = pf.tile([P, R], dt.bfloat16, tag="tt", bufs=2)
                nc.vector.scalar_tensor_tensor(
                    t_t[:], ev2_sb[:], r_sb[:, 2 * k : 2 * k + 1], ev_sb[:],
                    op0=ALU.mult, op1=ALU.max)
                w_t = pf.tile([P, R], dt.bfloat16, tag="wt", bufs=2)
                nc.vector.scalar_tensor_tensor(
                    w_t[:], t_t[:], eu_sb[:, 2 * k : 2 * k + 1], ct_t[:],
                    op0=ALU.mult, op1=ALU.mult)
                for vt in range(RT):
                    nc.tensor.matmul(
                        att_ps[vt][:],
                        w_t[:, vt * P : (vt + 1) * P],
                        zf_sb[:, k * O1 : (k + 1) * O1],
                        start=(k == 0), stop=(k == K - 1))
            # normalize + relu; keep [v, f] plus ones col for pooling
            for vt in range(RT):
                rec_sb = pf.tile([P, 1], dt.float32, tag="rec", bufs=2)
                nc.vector.reciprocal(rec_sb[:], att_ps[vt][:, OUT:O1])
                ox = pfo.tile([P, O1], dt.bfloat16, tag=f"ox{vt}", bufs=1)
                nc.vector.tensor_scalar(
                    ox[:, 0:OUT], att_ps[vt][:, 0:OUT], rec_sb[:], 0.0,
                    op0=ALU.mult, op1=ALU.max)
                nc.vector.memset(ox[:, OUT:O1], 1.0)
                outex[vt] = ox

          # ============= phase C: gated attention pooling ==================
          if True:
            outT_sb = pfo.tile([P, R], dt.bfloat16)
            with tc.tile_pool(name="phC_ps", bufs=1, space="PSUM") as pg_ps:
                for vt in range(RT):
                    trp = pg_ps.tile([P, P], dt.bfloat16, tag="tr", bufs=2)
                    nc.tensor.transpose(trp[:], outex[vt][:, 0:OUT], ident_sb[:])
                    nc.scalar.activation(
                        outT_sb[:, vt * P : (vt + 1) * P], trp[:], ACT.Copy)
            expa = [None] * RT
            with tc.tile_pool(name="phC", bufs=1) as pg, \
                 tc.tile_pool(name="phC2_ps", bufs=1, space="PSUM") as ph_ps:
                for vt in range(RT):
                    ab_ps = ph_ps.tile([P, 2 * D_ATT], dt.float32, tag="ab", bufs=2)
                    nc.tensor.matmul(
                        ab_ps[:], outT_sb[:, vt * P : (vt + 1) * P], wab_sb[:],
                        start=True, stop=True)
                    ab_sb = pg.tile([P, 2 * D_ATT], dt.float32, tag="absb", bufs=2)
                    nc.vector.tensor_tensor(ab_sb[:], ab_ps[:], babbc_sb[:], op=ALU.add)
                    tg_sb = pg.tile([P, 2 * D_ATT], dt.float32, tag="tg", bufs=2)
                    nc.scalar.activation(tg_sb[:, 0:D_ATT], ab_sb[:, 0:D_ATT], ACT.Tanh)
                    nc.scalar.activation(
                        tg_sb[:, D_ATT : 2 * D_ATT], ab_sb[:, D_ATT : 2 * D_ATT],
                        ACT.Sigmoid)
                    prod_sb = pg.tile([P, D_ATT], dt.float32, tag="prod", bufs=2)
                    nc.vector.tensor_tensor(
                        prod_sb[:], tg_sb[:, 0:D_ATT], tg_sb[:, D_ATT : 2 * D_ATT],
                        op=ALU.mult)
                    junk_sb = pg.tile([P, D_ATT], dt.float32, tag="junk", bufs=2)
                    acol_sb = pg.tile([P, 1], dt.float32, tag="acol", bufs=2)
                    nc.vector.scalar_tensor_tensor(
                        junk_sb[:], prod_sb[:], 1.0, wcbc_sb[:],
                        op0=ALU.mult, op1=ALU.mult, accum_out=acol_sb[:])
                    ea = pg.tile([P, 1], dt.bfloat16, tag=f"ea{vt}", bufs=1)
                    nc.scalar.activation(ea[:], acol_sb[:], ACT.Exp, bias=bcbc_sb[:])
                    expa[vt] = ea
                pool_ps = ph_ps.tile([1, O1], dt.float32, tag="pool", bufs=1)
                for vt in range(RT):
                    nc.tensor.matmul(
                        pool_ps[:], expa[vt][:], outex[vt][:],
                        start=(vt == 0), stop=(vt == RT - 1))
                pool_sb = pg.tile([1, O1], dt.float32)
                nc.vector.tensor_copy(pool_sb[:], pool_ps[:])
                nc.sync.dma_start(poolb[:], pool_sb[:])
                nc.gpsimd.collective_compute(
                    "AllReduce", ALU.add, replica_groups=groups,
                    ins=[poolb[:]], outs=[poolr[:]])
                polr_sb = pg.tile([1, O1], dt.float32)
                nc.sync.dma_start(polr_sb[:], poolr[:])
                rec2_sb = pg.tile([1, 1], dt.float32)
                nc.vector.reciprocal(rec2_sb[:], polr_sb[:, OUT:O1])
                res_sb = pg.tile([1, OUT], dt.float32)
                nc.vector.tensor_scalar(
                    res_sb[:], polr_sb[:, 0:OUT], rec2_sb[:], None, op0=ALU.mult)
                nc.sync.dma_start(out_ext[:], res_sb[:])

    nc.finalize()
    return nc


_PROGRAM = None


def _get_program():
    global _PROGRAM
    if _PROGRAM is None:
        _PROGRAM = build_program()
    return _PROGRAM


def prep_in_maps(x, G, src, dst, W1, b1, W2, b2, Wg, a_src, a_dst, Wa, ba, Wb, bb,
                 Wc, bc):
    x = np.asarray(x, np.float32)
    G = np.asarray(G, np.float32)
    src = np.asarray(src).astype(np.int64)
    dst = np.asarray(dst).astype(np.int64)

    # host-side prep (cheap relative to device work; no device data involved)
    GT = np.ascontiguousarray(G.T).astype(BF16)
    xT = np.ascontiguousarray(x.T).astype(BF16)
    C = np.zeros((N, N), np.float32)
    np.add.at(C, (src, dst), 1.0)
    C[np.arange(N), np.arange(N)] += 1.0
    Cb = C.astype(BF16)

    va = (np.asarray(Wg, np.float32) @ np.asarray(a_src, np.float32))
    vb = (np.asarray(Wg, np.float32) @ np.asarray(a_dst, np.float32))
    vab = np.stack([va, vb], 1).astype(BF16)
    wab = np.concatenate([np.asarray(Wa, np.float32),
                          np.asarray(Wb, np.float32)], 1).astype(BF16)
    bab = np.concatenate([np.asarray(ba, np.float32),
                          np.asarray(bb, np.float32)], 0)

    bcast = lambda v: np.broadcast_to(np.asarray(v, np.float32)[None, :],
                                      (P, len(np.asarray(v).reshape(-1)))).copy()
    common = {
        "w1": np.asarray(W1, np.float32).astype(BF16),
        "w2": np.asarray(W2, np.float32).astype(BF16),
        "wg": np.asarray(Wg, np.float32).astype(BF16),
        "vab": vab,
        "wab": wab,
        "b1bc": bcast(b1),
        "b2bc": bcast(b2),
        "babbc": bcast(bab),
        "wcbc": bcast(np.asarray(Wc, np.float32).reshape(-1)),
        "bcbc": np.full((P, 1), float(np.asarray(bc).reshape(-1)[0]), np.float32),
        "ident": np.eye(P, dtype=np.float32).astype(BF16),
    }
    in_maps = []
    for c in range(M):
        sl = slice(c * R, (c + 1) * R)
        in_maps.append({
            "gt": np.ascontiguousarray(GT[:, sl]),
            "ct": np.ascontiguousarray(Cb[:, sl]),
            "xt": np.ascontiguousarray(xT[:, sl]),
            **common,
        })
    return in_maps


def kernel(**inputs):
    from concourse.bass_utils import run_bass_kernel_spmd

    in_maps = prep_in_maps(**inputs)
    nc = _get_program()
    res = run_bass_kernel_spmd(nc, in_maps, list(range(M)))
    return np.asarray(res.results[0]["out"], np.float32)

